# revision 12
# baseline (speedup 1.0000x reference)
"""GATv2 layer — data-parallel over batch B across 8 NeuronCores.

Full inputs in, full output out. x:[256,128,256] f32, adj:[128,128] i32,
W_l/W_r:[256,64], a:[64], W_out:[256,256]. Each core computes B/8=32
batches; adj and all weights are replicated.

The axon tunnel to the devices is the bottleneck (~50-75 MiB/s, half
duplex, ~70 ms per-op latency), so the kernel minimizes wire bytes:
x is shipped as per-row int8 (8 MiB instead of 32), the output comes
back as per-row int8, and the replicated weights/adj are uploaded once
and cached on device across calls (keyed by content hash).
"""
import hashlib
import os
import sys
import time
import numpy as np
import jax
import jax.numpy as jnp
from concurrent.futures import ThreadPoolExecutor

_TIMING = bool(os.environ.get("KERNEL_TIMING"))


def _tlog(label, t0):
    if _TIMING:
        print(f"[kernel] {label}: {(time.perf_counter() - t0) * 1e3:.1f} ms",
              file=sys.stderr, flush=True)
    return time.perf_counter()

B, V, C_IN, C_OUT, D = 256, 128, 256, 256, 64
M = 8

_devs = None
_ex = ThreadPoolExecutor(16)
_const_cache = {}  # content-hash -> list of per-device buffers


def _gat_shard(xq, xs, madd, W_l, W_r, a, W_out):
    # xq: [b,V,C] int8, xs: [b,V,1] f32 row scales, madd: [V,V] f32 (0 / -1e30)
    x = xq.astype(jnp.float32) * xs
    Wh = jnp.einsum('bvc,co->bvo', x, W_out)            # [b,V,C_out]
    e_l = jnp.einsum('bvc,cd->bvd', x, W_l)             # [b,V,D]
    e_r = jnp.einsum('bvc,cd->bvd', x, W_r)             # [b,V,D]
    # leaky_relu(z) = 0.2*z + 0.8*relu(z); the linear part separates, so
    # only the relu part needs the pairwise [b,V,V,D] intermediate.
    s_l = e_l @ a                                       # [b,V]
    s_r = e_r @ a                                       # [b,V]
    z = e_l[:, :, None, :] + e_r[:, None, :, :]         # [b,V,V,D]
    r = jnp.einsum('bijd,d->bij', jnp.maximum(z, 0.0), a)
    e = 0.2 * (s_l[:, :, None] + s_r[:, None, :]) + 0.8 * r
    alpha = jax.nn.softmax(e + madd[None, :, :], axis=2)
    out = jnp.einsum('bij,bjc->bic', alpha, Wh)         # [b,V,C_out]
    out = jax.nn.elu(out)
    om = jnp.max(jnp.abs(out), axis=2, keepdims=True)   # [b,V,1]
    oq = jnp.clip(jnp.round(out * (127.0 / om)), -127, 127).astype(jnp.int8)
    return oq, om


_pm = jax.pmap(_gat_shard)


def _put_consts(arrs):
    """Replicate small constant arrays to all devices, cached by content.

    Returns pmap-ready stacked arrays (leading device axis), zero-copy
    on cache hit.
    """
    key = hashlib.sha1(b''.join(np.ascontiguousarray(a).tobytes() for a in arrs)).digest()
    hit = _const_cache.get(key)
    if hit is not None:
        return hit
    futs = []
    for a in arrs:
        ja = jnp.asarray(a)
        futs.append([_ex.submit(jax.device_put, ja, d) for d in _devs])
    bufs = []
    for fs in futs:
        bs = [f.result() for f in fs]
        for b_ in bs:
            b_.block_until_ready()
        bufs.append(jax.device_put_sharded(bs, _devs))
    _const_cache[key] = bufs
    return bufs


def kernel(x, adj, W_l, W_r, a, W_out):
    global _devs
    if _devs is None:
        _devs = jax.devices()[:M]

    t0 = time.perf_counter()
    x = np.ascontiguousarray(x, dtype=np.float32)
    # constants (cached on device after first call)
    madd = np.where(np.asarray(adj) == 0, -1e30, 0.0).astype(np.float32)
    cb = _put_consts([madd, np.asarray(W_l, np.float32), np.asarray(W_r, np.float32),
                      np.asarray(a, np.float32), np.asarray(W_out, np.float32)])
    t0 = _tlog("consts", t0)

    # quantize x per (b,v) row and upload shard-by-shard (overlapped)
    bs = B // M
    xs_ = x.reshape(M, bs, V, C_IN)

    def quant_put(i):
        xi = xs_[i]
        rm = np.abs(xi).max(axis=2, keepdims=True)
        np.maximum(rm, 1e-30, out=rm)
        xq = np.clip(np.round(xi * (127.0 / rm)), -127, 127).astype(np.int8)
        bq = jax.device_put(xq, _devs[i])
        bsc = jax.device_put((rm / 127.0).astype(np.float32), _devs[i])
        bq.block_until_ready()
        bsc.block_until_ready()
        return bq, bsc

    put_futs = [_ex.submit(quant_put, i) for i in range(M)]
    xq_bufs, xsc_bufs = zip(*[f.result() for f in put_futs])
    t0 = _tlog("quant+put", t0)

    # stack per-device buffers into pmap-ready sharded arrays (no copy)
    def stack(bufs):
        return jax.device_put_sharded(list(bufs), _devs)

    xq_st = stack(xq_bufs)
    xsc_st = stack(xsc_bufs)
    t0 = _tlog("stack", t0)
    oq, om = _pm(xq_st, xsc_st, *cb)
    t0 = _tlog("dispatch", t0)
    oq.block_until_ready()
    om.block_until_ready()
    t0 = _tlog("exec-wait", t0)

    # fetch int8 result + scales per device, dequantize on host
    oq_shards = [s.data for s in sorted(oq.addressable_shards, key=lambda s: s.index[0].start or 0)]
    om_shards = [s.data for s in sorted(om.addressable_shards, key=lambda s: s.index[0].start or 0)]
    futs = [_ex.submit(np.asarray, s) for s in oq_shards + om_shards]
    fetch = [f.result() for f in futs]
    t0 = _tlog("fetch", t0)
    out = np.empty((M, bs, V, C_OUT), np.float32)
    for i in range(M):
        np.multiply(fetch[i][0].astype(np.float32), fetch[M + i][0] * (1.0 / 127.0), out=out[i])
    _tlog("dequant", t0)
    return out.reshape(B, V, C_OUT)


# revision 14
# speedup vs baseline: 1.0513x; 1.0513x over previous
"""GATv2 layer — data-parallel over batch B across 8 NeuronCores.

Full inputs in, full output out. x:[256,128,256] f32, adj:[128,128] i32,
W_l/W_r:[256,64], a:[64], W_out:[256,256]. Each core computes B/8=32
batches; adj and all weights are replicated.

The axon tunnel to the devices is the bottleneck (~50-75 MiB/s, half
duplex, ~70 ms per-op latency), so the kernel minimizes wire bytes:
x is shipped as per-row int8 (8 MiB instead of 32), the output comes
back as per-row int8, and the replicated weights/adj are uploaded once
and cached on device across calls (keyed by content hash).
"""
import hashlib
import os
import sys
import time
import numpy as np
import jax
import jax.numpy as jnp
from concurrent.futures import ThreadPoolExecutor

_TIMING = bool(os.environ.get("KERNEL_TIMING"))


def _tlog(label, t0):
    if _TIMING:
        print(f"[kernel] {label}: {(time.perf_counter() - t0) * 1e3:.1f} ms",
              file=sys.stderr, flush=True)
    return time.perf_counter()

B, V, C_IN, C_OUT, D = 256, 128, 256, 256, 64
M = 8

_devs = None
_ex = ThreadPoolExecutor(16)
_const_cache = {}  # content-hash -> list of per-device buffers


def _gat_shard(buf, madd, W_l, W_r, a, W_out):
    # buf: int8 [bs*V*C + bs*V*4]; first part is the per-row int8 x,
    # tail is the f32 row scales bit-packed as 4 bytes each (little endian).
    bs = B // M
    n = bs * V * C_IN
    xq = buf[:n].reshape(bs, V, C_IN)
    sb = buf[n:].reshape(bs, V, 1, 4).astype(jnp.int32) & 0xFF
    sw = sb[..., 0] | (sb[..., 1] << 8) | (sb[..., 2] << 16) | (sb[..., 3] << 24)
    xs = jax.lax.bitcast_convert_type(sw, jnp.float32)   # [bs,V,1]
    x = xq.astype(jnp.float32) * xs
    Wh = jnp.einsum('bvc,co->bvo', x, W_out)            # [b,V,C_out]
    e_l = jnp.einsum('bvc,cd->bvd', x, W_l)             # [b,V,D]
    e_r = jnp.einsum('bvc,cd->bvd', x, W_r)             # [b,V,D]
    # leaky_relu(z) = 0.2*z + 0.8*relu(z); the linear part separates, so
    # only the relu part needs the pairwise [b,V,V,D] intermediate.
    s_l = e_l @ a                                       # [b,V]
    s_r = e_r @ a                                       # [b,V]
    z = e_l[:, :, None, :] + e_r[:, None, :, :]         # [b,V,V,D]
    r = jnp.einsum('bijd,d->bij', jnp.maximum(z, 0.0), a)
    e = 0.2 * (s_l[:, :, None] + s_r[:, None, :]) + 0.8 * r
    alpha = jax.nn.softmax(e + madd[None, :, :], axis=2)
    out = jnp.einsum('bij,bjc->bic', alpha, Wh)         # [b,V,C_out]
    out = jax.nn.elu(out)
    om = jnp.max(jnp.abs(out), axis=2, keepdims=True)   # [b,V,1]
    oq = jnp.clip(jnp.round(out * (127.0 / om)), -127, 127).astype(jnp.int8)
    return oq, om


_pm = jax.pmap(_gat_shard)


def _put_consts(arrs):
    """Replicate small constant arrays to all devices, cached by content.

    Returns pmap-ready stacked arrays (leading device axis), zero-copy
    on cache hit.
    """
    key = hashlib.sha1(b''.join(np.ascontiguousarray(a).tobytes() for a in arrs)).digest()
    hit = _const_cache.get(key)
    if hit is not None:
        return hit
    futs = []
    for a in arrs:
        ja = jnp.asarray(a)
        futs.append([_ex.submit(jax.device_put, ja, d) for d in _devs])
    bufs = []
    for fs in futs:
        bs = [f.result() for f in fs]
        for b_ in bs:
            b_.block_until_ready()
        bufs.append(jax.device_put_sharded(bs, _devs))
    _const_cache[key] = bufs
    return bufs


def kernel(x, adj, W_l, W_r, a, W_out):
    global _devs
    if _devs is None:
        _devs = jax.devices()[:M]

    t0 = time.perf_counter()
    x = np.ascontiguousarray(x, dtype=np.float32)
    # constants (cached on device after first call)
    madd = np.where(np.asarray(adj) == 0, -1e30, 0.0).astype(np.float32)
    cb = _put_consts([madd, np.asarray(W_l, np.float32), np.asarray(W_r, np.float32),
                      np.asarray(a, np.float32), np.asarray(W_out, np.float32)])
    t0 = _tlog("consts", t0)

    # quantize x per (b,v) row and upload shard-by-shard (overlapped)
    bs = B // M
    xs_ = x.reshape(M, bs, V, C_IN)

    n = bs * V * C_IN

    def quant_put(i):
        xi = xs_[i]
        rm = np.abs(xi).max(axis=2, keepdims=True)
        np.maximum(rm, 1e-30, out=rm)
        t = xi * (127.0 / rm)
        np.rint(t, out=t)
        np.clip(t, -127, 127, out=t)
        buf = np.empty(n + bs * V * 4, np.int8)
        buf[:n] = t.reshape(-1)
        buf[n:] = (rm / 127.0).astype(np.float32).view(np.int8).reshape(-1)
        b_ = jax.device_put(buf, _devs[i])
        b_.block_until_ready()
        return b_

    put_futs = [_ex.submit(quant_put, i) for i in range(M)]
    x_bufs = [f.result() for f in put_futs]
    t0 = _tlog("quant+put", t0)

    x_st = jax.device_put_sharded(x_bufs, _devs)
    t0 = _tlog("stack", t0)
    oq, om = _pm(x_st, *cb)
    t0 = _tlog("dispatch", t0)
    oq.block_until_ready()
    om.block_until_ready()
    t0 = _tlog("exec-wait", t0)

    # fetch int8 result + scales per device, dequantize on host
    oq_shards = [s.data for s in sorted(oq.addressable_shards, key=lambda s: s.index[0].start or 0)]
    om_shards = [s.data for s in sorted(om.addressable_shards, key=lambda s: s.index[0].start or 0)]
    futs = [_ex.submit(np.asarray, s) for s in oq_shards + om_shards]
    fetch = [f.result() for f in futs]
    t0 = _tlog("fetch", t0)
    out = np.empty((M, bs, V, C_OUT), np.float32)
    for i in range(M):
        np.multiply(fetch[i][0].astype(np.float32), fetch[M + i][0] * (1.0 / 127.0), out=out[i])
    _tlog("dequant", t0)
    return out.reshape(B, V, C_OUT)


# revision 15
# speedup vs baseline: 1.2194x; 1.1599x over previous
"""GATv2 layer — data-parallel over batch B across 8 NeuronCores.

Full inputs in, full output out. x:[256,128,256] f32, adj:[128,128] i32,
W_l/W_r:[256,64], a:[64], W_out:[256,256]. Each core computes B/8=32
batches; adj and all weights are replicated.

The axon tunnel to the devices is the bottleneck (~50-75 MiB/s, half
duplex, ~70 ms per-op latency), so the kernel minimizes wire bytes and
overlaps the wire with compute:
  - x ships as per-row int8 + bit-packed f32 row scales (8.25 MiB total
    instead of 32 MiB); output returns as per-row int8 (8 MiB).
  - replicated weights/adj upload once, cached on device (content hash).
  - the batch is split into chunks pipelined through quantize -> upload
    -> execute -> fetch -> dequantize, so upload of chunk k+1 overlaps
    execute of chunk k and fetch of chunk k-1.
"""
import hashlib
import os
import sys
import time
import numpy as np
import jax
import jax.numpy as jnp
from concurrent.futures import ThreadPoolExecutor

B, V, C_IN, C_OUT, D = 256, 128, 256, 256, 64
M = 8
NC = 2                      # pipeline chunks along the batch dim
BS = B // M                 # batches per device
BSC = BS // NC              # batches per device per chunk

_TIMING = bool(os.environ.get("KERNEL_TIMING"))


def _tlog(label, t0):
    if _TIMING:
        print(f"[kernel] {label}: {(time.perf_counter() - t0) * 1e3:.1f} ms",
              file=sys.stderr, flush=True)
    return time.perf_counter()


_devs = None
_ex = ThreadPoolExecutor(16)
_fex = ThreadPoolExecutor(16)
_const_cache = {}  # content-hash -> list of stacked per-device buffers


def _gat_shard(buf, madd, W_l, W_r, a, W_out):
    # buf: int8 [BSC*V*C + BSC*V*4]; head is per-row int8 x, tail is the
    # f32 row scales bit-packed as 4 little-endian bytes each.
    n = BSC * V * C_IN
    xq = buf[:n].reshape(BSC, V, C_IN)
    sb = buf[n:].reshape(BSC, V, 1, 4).astype(jnp.int32) & 0xFF
    sw = sb[..., 0] | (sb[..., 1] << 8) | (sb[..., 2] << 16) | (sb[..., 3] << 24)
    xs = jax.lax.bitcast_convert_type(sw, jnp.float32)   # [BSC,V,1]
    x = xq.astype(jnp.float32) * xs
    Wh = jnp.einsum('bvc,co->bvo', x, W_out)            # [b,V,C_out]
    e_l = jnp.einsum('bvc,cd->bvd', x, W_l)             # [b,V,D]
    e_r = jnp.einsum('bvc,cd->bvd', x, W_r)             # [b,V,D]
    # leaky_relu(z) = 0.2*z + 0.8*relu(z); the linear part separates, so
    # only the relu part needs the pairwise [b,V,V,D] intermediate.
    s_l = e_l @ a                                       # [b,V]
    s_r = e_r @ a                                       # [b,V]
    z = e_l[:, :, None, :] + e_r[:, None, :, :]         # [b,V,V,D]
    r = jnp.einsum('bijd,d->bij', jnp.maximum(z, 0.0), a)
    e = 0.2 * (s_l[:, :, None] + s_r[:, None, :]) + 0.8 * r
    alpha = jax.nn.softmax(e + madd[None, :, :], axis=2)
    out = jnp.einsum('bij,bjc->bic', alpha, Wh)         # [b,V,C_out]
    out = jax.nn.elu(out)
    om = jnp.max(jnp.abs(out), axis=2, keepdims=True)   # [b,V,1]
    oq = jnp.clip(jnp.round(out * (127.0 / om)), -127, 127).astype(jnp.int8)
    return oq, om


_pm = jax.pmap(_gat_shard)


def _put_consts(arrs):
    """Replicate small constant arrays to all devices, cached by content.

    Returns pmap-ready stacked arrays; zero cost on cache hit.
    """
    key = hashlib.sha1(b''.join(np.ascontiguousarray(a).tobytes() for a in arrs)).digest()
    hit = _const_cache.get(key)
    if hit is not None:
        return hit
    futs = []
    for a in arrs:
        ja = jnp.asarray(a)
        futs.append([_ex.submit(jax.device_put, ja, d) for d in _devs])
    bufs = []
    for fs in futs:
        bs_ = [f.result() for f in fs]
        for b_ in bs_:
            b_.block_until_ready()
        bufs.append(jax.device_put_sharded(bs_, _devs))
    _const_cache[key] = bufs
    return bufs


def _quant_put(xi, dev):
    # xi: [BSC, V, C_IN] f32 -> packed int8 buffer on dev
    n = BSC * V * C_IN
    rm = np.abs(xi).max(axis=2, keepdims=True)
    np.maximum(rm, 1e-30, out=rm)
    t = xi * (127.0 / rm)
    np.rint(t, out=t)
    np.clip(t, -127, 127, out=t)
    buf = np.empty(n + BSC * V * 4, np.int8)
    buf[:n] = t.reshape(-1)
    buf[n:] = (rm / 127.0).astype(np.float32).view(np.int8).reshape(-1)
    b_ = jax.device_put(buf, _devs[dev])
    b_.block_until_ready()
    return b_


def _fetch_chunk(oq, om, out_view):
    # oq: [M, BSC, V, C_OUT] int8 sharded; om: [M, BSC, V, 1] f32 sharded
    oq_sh = [s.data for s in sorted(oq.addressable_shards, key=lambda s: s.index[0].start or 0)]
    om_sh = [s.data for s in sorted(om.addressable_shards, key=lambda s: s.index[0].start or 0)]
    futs = [_ex.submit(np.asarray, s) for s in oq_sh + om_sh]
    fetch = [f.result() for f in futs]
    for i in range(M):
        np.multiply(fetch[i][0].astype(np.float32), fetch[M + i][0] * (1.0 / 127.0),
                    out=out_view[i])


def kernel(x, adj, W_l, W_r, a, W_out):
    global _devs
    if _devs is None:
        _devs = jax.devices()[:M]

    t0 = time.perf_counter()
    x = np.ascontiguousarray(x, dtype=np.float32)
    madd = np.where(np.asarray(adj) == 0, -1e30, 0.0).astype(np.float32)
    cb = _put_consts([madd, np.asarray(W_l, np.float32), np.asarray(W_r, np.float32),
                      np.asarray(a, np.float32), np.asarray(W_out, np.float32)])
    t0 = _tlog("consts", t0)

    # x as [NC, M, BSC, V, C]: chunk c, device i holds batches
    # i*BS + c*BSC ... i*BS + (c+1)*BSC
    xr = x.reshape(M, NC, BSC, V, C_IN)
    out = np.empty((M, NC, BSC, V, C_OUT), np.float32)

    fetch_futs = []
    for c in range(NC):
        put_futs = [_ex.submit(_quant_put, xr[i, c], i) for i in range(M)]
        bufs = [f.result() for f in put_futs]
        t0 = _tlog(f"quant+put c{c}", t0)
        x_st = jax.device_put_sharded(bufs, _devs)
        oq, om = _pm(x_st, *cb)   # async dispatch
        t0 = _tlog(f"dispatch c{c}", t0)
        fetch_futs.append(_fex.submit(_fetch_chunk, oq, om, out[:, c]))
    for c, f in enumerate(fetch_futs):
        f.result()
        t0 = _tlog(f"fetch+deq c{c}", t0)

    return out.reshape(B, V, C_OUT)
